# revision 1
# baseline (speedup 1.0000x reference)
"""Multi-head causal attention (B=4, S=2048, D=1024, H=16) on 8 TRN2 NeuronCores.

Sharding: data-parallel over batch (4) x tensor-parallel over heads (2 groups
of 8 heads) = 8 cores. Each core computes, for its (batch, head-group):
  Q^T/K^T = (x @ Wq/Wk)^T   [dc, S]   (dc = 512 head-group dims)
  V       = x @ Wv          [S, dc]
  per head h, per 512-wide query block iB (flash-style, scores transposed):
    E[j, i]      = exp(scoresT / 8) with causal mask (j <= i), j tiled by 128
    attnoutT|r   = [V_h | ones].T @ E   -> [65, i]  (row 64 = softmax denom r)
    anorm        = attnoutT * (1/r)      (broadcast over d)
  out_partial[i, :] += anorm_h.T @ (W_o[:, cols].T)   accumulated over heads
Host sums the two head-group partials per batch (the W_o row-shard
all-reduce from the sharding hint, done on host during unshard).

Matmuls run in float32r (full-rate fp32 PE mode, ~1.3e-4 rel err).
"""

import sys

if "/opt/trn_rl_repo" not in sys.path:
    sys.path.insert(0, "/opt/trn_rl_repo")

import numpy as np

import concourse.bacc as bacc
import concourse.mybir as mybir
import concourse.tile as tile
from concourse.bass import ts
from concourse.bass_utils import run_bass_kernel_spmd

F32 = mybir.dt.float32
F32R = mybir.dt.float32r
AF = mybir.ActivationFunctionType

B, S, D, H = 4, 2048, 1024, 16
HD = D // H           # 64
NCORES = 8
HG = 8                # heads per core
DC = HG * HD          # 512 feature cols per core
SB = 512              # s-block
NSB = S // SB         # 4
KC = D // 128         # 8 k-chunks
NIB = S // 512        # 4 query blocks
SCALE = 1.0 / np.sqrt(HD)

_cached_nc = None


def _build():
    nc = bacc.Bacc("TRN2", target_bir_lowering=False, debug=False)

    xt_d = nc.dram_tensor("xt", [D, S], F32R, kind="ExternalInput")      # x[b].T
    wq_d = nc.dram_tensor("wq", [D, DC], F32R, kind="ExternalInput")
    wk_d = nc.dram_tensor("wk", [D, DC], F32R, kind="ExternalInput")
    wv_d = nc.dram_tensor("wv", [D, DC], F32R, kind="ExternalInput")
    wot_d = nc.dram_tensor("wot", [DC, D], F32R, kind="ExternalInput")   # W_o[:, cols].T
    out_d = nc.dram_tensor("out", [S, D], F32, kind="ExternalOutput")

    with tile.TileContext(nc) as tc:
        with (
            tc.tile_pool(name="qtkt", bufs=4) as qtkt_pool,
            tc.tile_pool(name="vp", bufs=16) as v_pool,
            tc.tile_pool(name="ps_mm", bufs=2, space="PSUM") as ps_mm,
            tc.tile_pool(name="ps_acc", bufs=2, space="PSUM") as ps_acc,
            tc.tile_pool(name="ps_proj", bufs=2, space="PSUM") as ps_proj,
        ):
            # persistent tiles
            qt = [qtkt_pool.tile([128, S], F32R, tag="qt", name=f"qt{i}") for i in range(4)]
            kt = [qtkt_pool.tile([128, S], F32R, tag="kt", name=f"kt{i}") for i in range(4)]
            vt = [v_pool.tile([128, HG, HD + 1], F32R, tag="vt", name=f"vt{i}") for i in range(16)]
            ones8 = v_pool.tile([128, HG], F32, tag="ones8", bufs=1)
            nc.vector.memset(ones8, 1.0)

            # ---------------- Phase 1: QKV projections ----------------
            with (
                tc.tile_pool(name="wp", bufs=1) as w_pool,
                tc.tile_pool(name="xtp", bufs=2) as xt_pool,
            ):
                wq = w_pool.tile([128, KC, DC], F32R, tag="wq")
                wk = w_pool.tile([128, KC, DC], F32R, tag="wk")
                wv = w_pool.tile([128, KC, DC], F32R, tag="wv")
                # split loads per k-chunk so the first accumulation starts as
                # soon as chunk 0 lands instead of after the whole 2MB tensor
                for kc in range(KC):
                    nc.sync.dma_start(out=wq[:, kc, :], in_=wq_d[ts(kc, 128), :])
                    nc.sync.dma_start(out=wk[:, kc, :], in_=wk_d[ts(kc, 128), :])
                    nc.sync.dma_start(out=wv[:, kc, :], in_=wv_d[ts(kc, 128), :])

                for sb in range(NSB):
                    xt_t = xt_pool.tile([128, KC, SB], F32R, tag="xt")
                    for kc in range(KC):
                        nc.sync.dma_start(
                            out=xt_t[:, kc, :],
                            in_=xt_d[ts(kc, 128), ts(sb, SB)],
                        )
                    # Q^T, K^T: [dc-chunk(128), s-block] accumulated over k
                    for w_t, dst in ((wq, qt), (wk, kt)):
                        for m in range(4):
                            ps = ps_mm.tile([128, SB], F32, tag="mm")
                            for kc in range(KC):
                                nc.tensor.matmul(
                                    ps,
                                    w_t[:, kc, ts(m, 128)],
                                    xt_t[:, kc, :],
                                    start=(kc == 0), stop=(kc == KC - 1),
                                )
                            nc.vector.tensor_copy(dst[m][:, ts(sb, SB)], ps)
                    # V natural [s-chunk(128), dc] accumulated over k
                    for sc in range(4):
                        ps = ps_mm.tile([128, DC], F32, tag="mm")
                        for kc in range(KC):
                            nc.tensor.matmul(
                                ps,
                                xt_t[:, kc, ts(sc, 128)],
                                wv[:, kc, :],
                                start=(kc == 0), stop=(kc == KC - 1),
                            )
                        v_t = vt[4 * sb + sc]
                        nc.vector.tensor_copy(
                            v_t[:, :, 0:HD],
                            ps.rearrange("p (h d) -> p h d", h=HG),
                        )
                        nc.vector.tensor_copy(v_t[:, :, HD:HD + 1], ones8)

            # ---------------- Phase 2+3: attention + projection ----------------
            with (
                tc.tile_pool(name="wotp", bufs=8) as wot_pool,
                tc.tile_pool(name="ep", bufs=4) as e_pool,
                tc.tile_pool(name="t65p", bufs=4) as t65_pool,
                tc.tile_pool(name="anp", bufs=6) as an_pool,
                tc.tile_pool(name="rp", bufs=4) as r_pool,
                tc.tile_pool(name="bcp", bufs=3) as bc_pool,
                tc.tile_pool(name="op", bufs=2) as o_pool,
            ):
                wot = [wot_pool.tile([128, D], F32R, tag="wot", name=f"wot{i}") for i in range(4)]
                for t in range(4):
                    nc.sync.dma_start(out=wot[t], in_=wot_d[ts(t, 128), :])
                # junk bf16 weights: dependency-free LDWEIGHTS emitted between
                # real matmuls keep the PE activity monitor from down-clocking
                junk16 = wot_pool.tile([128, 128], mybir.dt.bfloat16, tag="junk", bufs=1)
                nc.vector.memset(junk16, 0.0)
                for iB in range(NIB):
                    njj = 4 * iB + 4
                    nu = njj // 2          # jj pairs
                    anorms = [None] * 4
                    for hp in range(HG // 2):
                        h0, h1 = 2 * hp, 2 * hp + 1
                        m = hp
                        accs = {h0: ps_acc.tile([128, SB], F32, tag="acc", name=f"acc0_{iB}_{hp}"),
                                h1: ps_acc.tile([128, SB], F32, tag="acc", name=f"acc1_{iB}_{hp}")}
                        for u in range(nu):
                            jj0, jj1 = 2 * u, 2 * u + 1
                            pss = {}
                            for h in (h0, h1):
                                rb = (h % 2) * 64
                                ps2 = ps_mm.tile([128, 1024], F32, tag="mm",
                                                 name=f"ps_{iB}_{hp}_{u}_{h}")
                                pss[h] = ps2
                                for q, jj in ((0, jj0), (1, jj1)):
                                    nc.tensor.matmul(
                                        ps2[:, ts(q, SB)],
                                        kt[m][rb:rb + 64, ts(jj, 128)],
                                        qt[m][rb:rb + 64, ts(iB, SB)],
                                        start=True, stop=True,
                                    )
                            nc.tensor.ldweights(junk16)
                            for h in (h0, h1):
                                e_t = e_pool.tile([128, 1024], F32R, tag="e",
                                                  name=f"e_{iB}_{hp}_{u}_{h}")
                                nc.scalar.activation(e_t, pss[h], AF.Exp,
                                                     scale=float(SCALE))
                                t0 = jj0 - 4 * iB
                                if t0 >= 0:
                                    nc.gpsimd.affine_select(
                                        out=e_t, in_=e_t,
                                        pattern=[[-128, 2], [1, SB]],
                                        compare_op=mybir.AluOpType.is_ge,
                                        fill=0.0, base=-128 * t0,
                                        channel_multiplier=-1,
                                    )
                                for q, jj in ((0, jj0), (1, jj1)):
                                    nc.tensor.matmul(
                                        accs[h][0:HD + 1, :],
                                        vt[jj][:, h, :],
                                        e_t[:, ts(q, SB)],
                                        start=(u == 0 and q == 0),
                                        stop=(u == nu - 1 and q == 1),
                                    )
                                nc.tensor.ldweights(junk16)
                        an_pair = an_pool.tile([128, SB], F32R, tag="an",
                                               name=f"an_{iB}_{hp}")
                        anorms[hp] = an_pair
                        for h in (h0, h1):
                            t65 = t65_pool.tile([HD + 1, SB], F32, tag="t65",
                                                name=f"t65_{iB}_{h}")
                            nc.vector.tensor_copy(t65, accs[h][0:HD + 1, :])
                            r_t = r_pool.tile([1, SB], F32, tag="r", name=f"r_{iB}_{h}")
                            nc.sync.dma_start(out=r_t, in_=t65[HD:HD + 1, :])
                            rec_t = r_pool.tile([1, SB], F32, tag="rec", name=f"rec_{iB}_{h}")
                            nc.vector.reciprocal_approx_fast(out=rec_t, in_=r_t)
                            bc_t = bc_pool.tile([HD, SB], F32, tag="bc", name=f"bc_{iB}_{h}")
                            nc.gpsimd.partition_broadcast(bc_t, rec_t)
                            if h == h0:
                                nc.vector.tensor_mul(an_pair[0:HD, :],
                                                     t65[0:HD, :], bc_t)
                            else:
                                an_odd = an_pool.tile([HD, SB], F32R, tag="anodd",
                                                      name=f"anodd_{iB}_{h}")
                                nc.vector.tensor_mul(an_odd, t65[0:HD, :], bc_t)
                                # stage odd head to partitions 64..127 (DMA can
                                # move across partitions; DVE cannot)
                                nc.sync.dma_start(out=an_pair[HD:128, :], in_=an_odd)

                    # output projection for this query block, summed over heads
                    for ic in range(4):
                        o_t = o_pool.tile([128, D], F32, tag="o")
                        for dh in range(2):
                            po = ps_proj.tile([128, 512], F32, tag="po")
                            for hp2 in range(4):
                                nc.tensor.matmul(
                                    po,
                                    anorms[hp2][:, ts(ic, 128)],
                                    wot[hp2][:, ts(dh, 512)],
                                    start=(hp2 == 0), stop=(hp2 == 3),
                                )
                            nc.tensor.ldweights(junk16)
                            nc.vector.tensor_copy(o_t[:, ts(dh, 512)], po)
                        nc.sync.dma_start(
                            out=out_d[iB * SB + ic * 128:iB * SB + (ic + 1) * 128, :],
                            in_=o_t,
                        )

    nc.compile()
    return nc


def kernel(x, W_q, W_k, W_v, W_o):
    global _cached_nc
    if _cached_nc is None:
        _cached_nc = _build()
    nc = _cached_nc

    x = np.asarray(x, dtype=np.float32)
    W_q = np.asarray(W_q, dtype=np.float32)
    W_k = np.asarray(W_k, dtype=np.float32)
    W_v = np.asarray(W_v, dtype=np.float32)
    W_o = np.asarray(W_o, dtype=np.float32)

    in_maps = []
    for c in range(NCORES):
        b, g = c // 2, c % 2
        cols = slice(g * DC, (g + 1) * DC)
        in_maps.append({
            "xt": np.ascontiguousarray(x[b].T),
            "wq": np.ascontiguousarray(W_q[:, cols]),
            "wk": np.ascontiguousarray(W_k[:, cols]),
            "wv": np.ascontiguousarray(W_v[:, cols]),
            "wot": np.ascontiguousarray(W_o[:, cols].T),
        })

    res = run_bass_kernel_spmd(nc, in_maps, list(range(NCORES))).results
    out = np.empty((B, S, D), np.float32)
    for b in range(B):
        out[b] = res[2 * b]["out"] + res[2 * b + 1]["out"]
    return out



# revision 6
# speedup vs baseline: 1.1260x; 1.1260x over previous
"""Multi-head causal attention (B=4, S=2048, D=1024, H=16) on 8 TRN2 NeuronCores.

Sharding: data-parallel over batch (4) x tensor-parallel over heads (2 groups
of 8 heads) = 8 cores. Each core computes, for its (batch, head-group):
  Q^T/K^T = (x @ Wq/Wk)^T   [dc, S]   (dc = 512 head-group dims)
  V       = x @ Wv          [S, dc]
  per head h, per 512-wide query block iB (flash-style, scores transposed):
    E[j, i]      = exp(scoresT / 8) with causal mask (j <= i), j tiled by 128
    attnoutT|r   = [V_h | ones].T @ E  -> [65, i]  (row 64 = softmax denom r)
    rec          = 1/r  (DVE, read directly from PSUM row 64)
    bc           = partition_broadcast(rec) -> [64, i]  (gpsimd, SBUF)
    anorm        = attnoutT * bc   (DVE, PSUM x SBUF -> SBUF)
  out_partial[i, :] += anorm_h.T @ (W_o[:, cols].T)   accumulated over heads
Host sums the two head-group partials per batch (the W_o row-shard
all-reduce from the sharding hint, done on host during unshard).

Q/K/V/E and the QKV projections run in bf16 (halves DMA + SBUF traffic,
enables fast weight loads); PSUM accumulation is fp32; the output
projection runs in fp32r for precision at the last layer.
"""

import sys

if "/opt/trn_rl_repo" not in sys.path:
    sys.path.insert(0, "/opt/trn_rl_repo")

import ml_dtypes
import numpy as np

import concourse.bacc as bacc
import concourse.mybir as mybir
import concourse.tile as tile
from concourse.bass import ts
from concourse.bass_utils import run_bass_kernel_spmd

F32 = mybir.dt.float32
F32R = mybir.dt.float32r
BF16 = mybir.dt.bfloat16
AF = mybir.ActivationFunctionType
BFNP = np.dtype(ml_dtypes.bfloat16)

B, S, D, H = 4, 2048, 1024, 16
HD = D // H           # 64
NCORES = 8
HG = 8                # heads per core
DC = HG * HD          # 512 feature cols per core
SB = 512              # s-block
NSB = S // SB         # 4
KC = D // 128         # 8 k-chunks
NIB = S // 512        # 4 query blocks
SCALE = 1.0 / np.sqrt(HD)

_cached_nc = None


def _build():
    nc = bacc.Bacc("TRN2", target_bir_lowering=False, debug=False)

    xt_d = nc.dram_tensor("xt", [D, S], BF16, kind="ExternalInput")      # x[b].T
    wq_d = nc.dram_tensor("wq", [D, DC], BF16, kind="ExternalInput")
    wk_d = nc.dram_tensor("wk", [D, DC], BF16, kind="ExternalInput")
    wv_d = nc.dram_tensor("wv", [D, DC], BF16, kind="ExternalInput")
    wot_d = nc.dram_tensor("wot", [DC, D], F32R, kind="ExternalInput")   # W_o[:, cols].T
    out_d = nc.dram_tensor("out", [S, D], F32, kind="ExternalOutput")

    with tile.TileContext(nc) as tc:
        with (
            tc.tile_pool(name="qtkt", bufs=4) as qtkt_pool,
            tc.tile_pool(name="vp", bufs=16) as v_pool,
            tc.tile_pool(name="ps_mm", bufs=2, space="PSUM") as ps_mm,
            tc.tile_pool(name="ps_acc", bufs=2, space="PSUM") as ps_acc,
            tc.tile_pool(name="ps_proj", bufs=2, space="PSUM") as ps_proj,
        ):
            # persistent tiles
            qt = [qtkt_pool.tile([128, S], BF16, tag="qt", name=f"qt{i}") for i in range(4)]
            kt = [qtkt_pool.tile([128, S], BF16, tag="kt", name=f"kt{i}") for i in range(4)]
            vt = [v_pool.tile([128, HG, HD + 1], BF16, tag="vt", name=f"vt{i}") for i in range(16)]
            ones8 = v_pool.tile([128, HG], BF16, tag="ones8", bufs=1)
            nc.vector.memset(ones8, 1.0)

            # ---------------- Phase 1: QKV projections ----------------
            with (
                tc.tile_pool(name="wp", bufs=1) as w_pool,
                tc.tile_pool(name="xtp", bufs=2) as xt_pool,
            ):
                wq = w_pool.tile([128, KC, DC], BF16, tag="wq")
                wk = w_pool.tile([128, KC, DC], BF16, tag="wk")
                wv = w_pool.tile([128, KC, DC], BF16, tag="wv")
                # split loads per k-chunk so the first accumulation starts as
                # soon as chunk 0 lands instead of after the whole tensor
                for kc in range(KC):
                    nc.sync.dma_start(out=wq[:, kc, :], in_=wq_d[ts(kc, 128), :])
                    nc.sync.dma_start(out=wk[:, kc, :], in_=wk_d[ts(kc, 128), :])
                    nc.sync.dma_start(out=wv[:, kc, :], in_=wv_d[ts(kc, 128), :])

                for sb in range(NSB):
                    xt_t = xt_pool.tile([128, KC, SB], BF16, tag="xt")
                    for kc in range(KC):
                        nc.sync.dma_start(
                            out=xt_t[:, kc, :],
                            in_=xt_d[ts(kc, 128), ts(sb, SB)],
                        )
                    # Q^T, K^T: [dc-chunk(128), s-block] accumulated over k
                    for w_t, dst in ((wq, qt), (wk, kt)):
                        for m in range(4):
                            ps = ps_mm.tile([128, SB], F32, tag="mm")
                            for kc in range(KC):
                                nc.tensor.matmul(
                                    ps,
                                    w_t[:, kc, ts(m, 128)],
                                    xt_t[:, kc, :],
                                    start=(kc == 0), stop=(kc == KC - 1),
                                )
                            nc.vector.tensor_copy(dst[m][:, ts(sb, SB)], ps)
                    # V natural [s-chunk(128), dc] accumulated over k
                    for sc in range(4):
                        ps = ps_mm.tile([128, DC], F32, tag="mm")
                        for kc in range(KC):
                            nc.tensor.matmul(
                                ps,
                                xt_t[:, kc, ts(sc, 128)],
                                wv[:, kc, :],
                                start=(kc == 0), stop=(kc == KC - 1),
                            )
                        v_t = vt[4 * sb + sc]
                        nc.vector.tensor_copy(
                            v_t[:, :, 0:HD],
                            ps.rearrange("p (h d) -> p h d", h=HG),
                        )
                        nc.vector.tensor_copy(v_t[:, :, HD:HD + 1], ones8)

            # ---------------- Phase 2+3: attention + projection ----------------
            with (
                tc.tile_pool(name="wotp", bufs=8) as wot_pool,
                tc.tile_pool(name="ep", bufs=4) as e_pool,
                tc.tile_pool(name="recp", bufs=4) as rec_pool,
                tc.tile_pool(name="anp", bufs=6) as an_pool,
                tc.tile_pool(name="bcp", bufs=4) as bc_pool,
                tc.tile_pool(name="op", bufs=2) as o_pool,
            ):
                wot = [wot_pool.tile([128, D], F32R, tag="wot", name=f"wot{i}") for i in range(4)]
                for t in range(4):
                    nc.sync.dma_start(out=wot[t], in_=wot_d[ts(t, 128), :])
                # junk bf16 weights: dependency-free LDWEIGHTS emitted between
                # real matmuls keep the PE activity monitor from down-clocking
                junk16 = wot_pool.tile([128, 8], BF16, tag="junk", bufs=1)
                nc.vector.memset(junk16, 0.0)
                for iB in range(NIB):
                    njj = 4 * iB + 4
                    nu = njj // 2          # jj pairs
                    anorms = [None] * 4
                    for hp in range(HG // 2):
                        h0, h1 = 2 * hp, 2 * hp + 1
                        m = hp
                        accs = {h0: ps_acc.tile([128, SB], F32, tag="acc", name=f"acc0_{iB}_{hp}"),
                                h1: ps_acc.tile([128, SB], F32, tag="acc", name=f"acc1_{iB}_{hp}")}
                        for u in range(nu):
                            jj0, jj1 = 2 * u, 2 * u + 1
                            pss = {}
                            for h in (h0, h1):
                                rb = (h % 2) * 64
                                ps2 = ps_mm.tile([128, 1024], F32, tag="mm",
                                                 name=f"ps_{iB}_{hp}_{u}_{h}")
                                pss[h] = ps2
                                for q, jj in ((0, jj0), (1, jj1)):
                                    nc.tensor.matmul(
                                        ps2[:, ts(q, SB)],
                                        kt[m][rb:rb + 64, ts(jj, 128)],
                                        qt[m][rb:rb + 64, ts(iB, SB)],
                                        start=True, stop=True,
                                    )
                            nc.tensor.ldweights(junk16)
                            for h in (h0, h1):
                                e_t = e_pool.tile([128, 1024], BF16, tag="e",
                                                  name=f"e_{iB}_{hp}_{u}_{h}")
                                nc.scalar.activation(e_t, pss[h], AF.Exp,
                                                     scale=float(SCALE))
                                t0 = jj0 - 4 * iB
                                if t0 >= 0:
                                    nc.gpsimd.affine_select(
                                        out=e_t, in_=e_t,
                                        pattern=[[-128, 2], [1, SB]],
                                        compare_op=mybir.AluOpType.is_ge,
                                        fill=0.0, base=-128 * t0,
                                        channel_multiplier=-1,
                                    )
                                for q, jj in ((0, jj0), (1, jj1)):
                                    nc.tensor.matmul(
                                        accs[h][0:HD + 1, :],
                                        vt[jj][:, h, :],
                                        e_t[:, ts(q, SB)],
                                        start=(u == 0 and q == 0),
                                        stop=(u == nu - 1 and q == 1),
                                    )
                                nc.tensor.ldweights(junk16)
                        an_pair = an_pool.tile([128, SB], F32R, tag="an",
                                               name=f"an_{iB}_{hp}")
                        anorms[hp] = an_pair
                        for h in (h0, h1):
                            t65 = rec_pool.tile([HD + 1, SB], F32, tag="t65",
                                                name=f"t65_{iB}_{h}")
                            nc.vector.tensor_copy(t65, accs[h][0:HD + 1, :])
                            r_t = rec_pool.tile([1, SB], F32, tag="r",
                                                name=f"r_{iB}_{h}")
                            nc.sync.dma_start(out=r_t, in_=t65[HD:HD + 1, :])
                            rec_t = rec_pool.tile([1, SB], F32, tag="rec",
                                                  name=f"rec_{iB}_{h}")
                            nc.vector.reciprocal_approx_fast(out=rec_t, in_=r_t)
                            bc_t = bc_pool.tile([HD, SB], F32, tag="bc",
                                                name=f"bc_{iB}_{h}")
                            nc.gpsimd.partition_broadcast(bc_t, rec_t)
                            if h == h0:
                                nc.vector.tensor_mul(an_pair[0:HD, :],
                                                     t65[0:HD, :], bc_t)
                            else:
                                an_odd = an_pool.tile([HD, SB], F32R, tag="anodd",
                                                      name=f"anodd_{iB}_{h}")
                                nc.vector.tensor_mul(an_odd, t65[0:HD, :], bc_t)
                                # stage odd head to partitions 64..127 (DMA can
                                # move across partitions; DVE cannot)
                                nc.sync.dma_start(out=an_pair[HD:128, :], in_=an_odd)

                    # output projection for this query block, summed over heads
                    for ic in range(4):
                        o_t = o_pool.tile([128, D], F32, tag="o")
                        for dh in range(2):
                            po = ps_proj.tile([128, 512], F32, tag="po")
                            for hp2 in range(4):
                                nc.tensor.matmul(
                                    po,
                                    anorms[hp2][:, ts(ic, 128)],
                                    wot[hp2][:, ts(dh, 512)],
                                    start=(hp2 == 0), stop=(hp2 == 3),
                                )
                            nc.tensor.ldweights(junk16)
                            nc.vector.tensor_copy(o_t[:, ts(dh, 512)], po)
                        nc.sync.dma_start(
                            out=out_d[iB * SB + ic * 128:iB * SB + (ic + 1) * 128, :],
                            in_=o_t,
                        )

    nc.compile()
    return nc


def make_in_maps(x, W_q, W_k, W_v, W_o):
    x = np.asarray(x, dtype=np.float32)
    W_q = np.asarray(W_q, dtype=np.float32)
    W_k = np.asarray(W_k, dtype=np.float32)
    W_v = np.asarray(W_v, dtype=np.float32)
    W_o = np.asarray(W_o, dtype=np.float32)

    in_maps = []
    for c in range(NCORES):
        b, g = c // 2, c % 2
        cols = slice(g * DC, (g + 1) * DC)
        in_maps.append({
            "xt": np.ascontiguousarray(x[b].T).astype(BFNP),
            "wq": np.ascontiguousarray(W_q[:, cols]).astype(BFNP),
            "wk": np.ascontiguousarray(W_k[:, cols]).astype(BFNP),
            "wv": np.ascontiguousarray(W_v[:, cols]).astype(BFNP),
            "wot": np.ascontiguousarray(W_o[:, cols].T),
        })
    return in_maps


def kernel(x, W_q, W_k, W_v, W_o):
    global _cached_nc
    if _cached_nc is None:
        _cached_nc = _build()
    nc = _cached_nc

    in_maps = make_in_maps(x, W_q, W_k, W_v, W_o)
    res = run_bass_kernel_spmd(nc, in_maps, list(range(NCORES))).results
    out = np.empty((B, S, D), np.float32)
    for b in range(B):
        out[b] = res[2 * b]["out"] + res[2 * b + 1]["out"]
    return out


# revision 7
# speedup vs baseline: 1.2099x; 1.0745x over previous
"""Multi-head causal attention (B=4, S=2048, D=1024, H=16) on 8 TRN2 NeuronCores.

Sharding: data-parallel over batch (4) x tensor-parallel over heads (2 groups
of 8 heads) = 8 cores. Each core computes, for its (batch, head-group):
  Q^T/K^T = (x @ Wq/Wk)^T   [dc, S]   (dc = 512 head-group dims)
  V       = x @ Wv          [S, dc]
  per head h, per 512-wide query block iB (flash-style, scores transposed):
    E[j, i]      = exp(scoresT / 8) with causal mask (j <= i), j tiled by 128
    attnoutT|r   = [V_h | ones].T @ E  -> [65, i]  (row 64 = softmax denom r)
    anorm        = attnoutT * (1/r)    (broadcast over d)
  out_partial[i, :] += anorm_h.T @ (W_o[:, cols].T)   accumulated over heads
Host sums the two head-group partials per batch (the W_o row-shard
all-reduce from the sharding hint, done on host during unshard).

Schedule: projection block sb and attention query-block iB=sb are
interleaved — causal attention for queries [512*sb, 512*(sb+1)) needs
exactly the K/V produced by projection blocks <= sb.  This keeps the PE
dense (projection matmuls fill the exp-bound gaps of early attention
blocks) so the HAM activity monitor never re-throttles the PE clock.

The last two key chunks of each diagonal block are computed only for the
query half that can attend to them (staircase), halving their score/exp/
attend cost.

Q/K/V/E and the QKV projections run in bf16 (halves DMA + SBUF traffic,
enables fast weight loads); PSUM accumulation is fp32; the output
projection runs in fp32r for precision at the last layer.
"""

import sys

if "/opt/trn_rl_repo" not in sys.path:
    sys.path.insert(0, "/opt/trn_rl_repo")

import ml_dtypes
import numpy as np

import concourse.bacc as bacc
import concourse.mybir as mybir
import concourse.tile as tile
from concourse.bass import ts
from concourse.bass_utils import run_bass_kernel_spmd

F32 = mybir.dt.float32
F32R = mybir.dt.float32r
BF16 = mybir.dt.bfloat16
AF = mybir.ActivationFunctionType
BFNP = np.dtype(ml_dtypes.bfloat16)

B, S, D, H = 4, 2048, 1024, 16
HD = D // H           # 64
NCORES = 8
HG = 8                # heads per core
DC = HG * HD          # 512 feature cols per core
SB = 512              # s-block
NSB = S // SB         # 4
KC = D // 128         # 8 k-chunks
NIB = S // 512        # 4 query blocks
SCALE = 1.0 / np.sqrt(HD)

_cached_nc = None


def _build():
    nc = bacc.Bacc("TRN2", target_bir_lowering=False, debug=False)

    xt_d = nc.dram_tensor("xt", [D, S], BF16, kind="ExternalInput")      # x[b].T
    wq_d = nc.dram_tensor("wq", [D, DC], BF16, kind="ExternalInput")
    wk_d = nc.dram_tensor("wk", [D, DC], BF16, kind="ExternalInput")
    wv_d = nc.dram_tensor("wv", [D, DC], BF16, kind="ExternalInput")
    wot_d = nc.dram_tensor("wot", [DC, D], F32R, kind="ExternalInput")   # W_o[:, cols].T
    out_d = nc.dram_tensor("out", [S, D], F32, kind="ExternalOutput")

    with tile.TileContext(nc) as tc:
        with (
            tc.tile_pool(name="qtkt", bufs=4) as qtkt_pool,
            tc.tile_pool(name="vp", bufs=16) as v_pool,
            tc.tile_pool(name="wp", bufs=1) as w_pool,
            tc.tile_pool(name="xtp", bufs=2) as xt_pool,
            tc.tile_pool(name="wotp", bufs=8) as wot_pool,
            tc.tile_pool(name="ep", bufs=4) as e_pool,
            tc.tile_pool(name="recp", bufs=4) as rec_pool,
            tc.tile_pool(name="anp", bufs=6) as an_pool,
            tc.tile_pool(name="bcp", bufs=4) as bc_pool,
            tc.tile_pool(name="op", bufs=2) as o_pool,
            tc.tile_pool(name="ps_mm", bufs=2, space="PSUM") as ps_mm,
            tc.tile_pool(name="ps_acc", bufs=2, space="PSUM") as ps_acc,
            tc.tile_pool(name="ps_proj", bufs=2, space="PSUM") as ps_proj,
        ):
            # persistent tiles
            qt = [qtkt_pool.tile([128, S], BF16, tag="qt", name=f"qt{i}") for i in range(4)]
            kt = [qtkt_pool.tile([128, S], BF16, tag="kt", name=f"kt{i}") for i in range(4)]
            vt = [v_pool.tile([128, HG, HD + 1], BF16, tag="vt", name=f"vt{i}") for i in range(16)]
            ones8 = v_pool.tile([128, HG], BF16, tag="ones8", bufs=1)
            nc.vector.memset(ones8, 1.0)

            wot = [wot_pool.tile([128, D], F32R, tag="wot", name=f"wot{i}") for i in range(4)]
            # junk bf16 weights: dependency-free LDWEIGHTS emitted between
            # real matmuls keep the PE activity monitor from down-clocking
            junk16 = wot_pool.tile([128, 8], BF16, tag="junk", bufs=1)
            nc.vector.memset(junk16, 0.0)

            wq = w_pool.tile([128, KC, DC], BF16, tag="wq")
            wk = w_pool.tile([128, KC, DC], BF16, tag="wk")
            wv = w_pool.tile([128, KC, DC], BF16, tag="wv")
            # first projection block's x lands first, interleaved per k-chunk
            # with the weights it multiplies, so the PE starts within ~2us
            xt0 = xt_pool.tile([128, KC, SB], BF16, tag="xt", name="xt_sb0")
            for kc in range(KC):
                nc.sync.dma_start(out=xt0[:, kc, :], in_=xt_d[ts(kc, 128), ts(0, SB)])
                nc.sync.dma_start(out=wq[:, kc, :], in_=wq_d[ts(kc, 128), :])
                nc.sync.dma_start(out=wk[:, kc, :], in_=wk_d[ts(kc, 128), :])
                nc.sync.dma_start(out=wv[:, kc, :], in_=wv_d[ts(kc, 128), :])
            for t in range(4):
                nc.sync.dma_start(out=wot[t], in_=wot_d[ts(t, 128), :])

            for sb in range(NSB):
                # ---------- projection block sb ----------
                if sb == 0:
                    xt_t = xt0
                else:
                    xt_t = xt_pool.tile([128, KC, SB], BF16, tag="xt",
                                        name=f"xt_sb{sb}")
                    for kc in range(KC):
                        nc.sync.dma_start(
                            out=xt_t[:, kc, :],
                            in_=xt_d[ts(kc, 128), ts(sb, SB)],
                        )
                # Q^T, K^T: [dc-chunk(128), s-block] accumulated over k
                for w_t, dst in ((wq, qt), (wk, kt)):
                    for m in range(4):
                        ps = ps_mm.tile([128, SB], F32, tag="mm")
                        for kc in range(KC):
                            nc.tensor.matmul(
                                ps,
                                w_t[:, kc, ts(m, 128)],
                                xt_t[:, kc, :],
                                start=(kc == 0), stop=(kc == KC - 1),
                            )
                        nc.vector.tensor_copy(dst[m][:, ts(sb, SB)], ps)
                # V natural [s-chunk(128), dc] accumulated over k
                for sc in range(4):
                    ps = ps_mm.tile([128, DC], F32, tag="mm")
                    for kc in range(KC):
                        nc.tensor.matmul(
                            ps,
                            xt_t[:, kc, ts(sc, 128)],
                            wv[:, kc, :],
                            start=(kc == 0), stop=(kc == KC - 1),
                        )
                    v_t = vt[4 * sb + sc]
                    nc.vector.tensor_copy(
                        v_t[:, :, 0:HD],
                        ps.rearrange("p (h d) -> p h d", h=HG),
                    )
                    nc.vector.tensor_copy(v_t[:, :, HD:HD + 1], ones8)

                # ---------- attention query block iB = sb ----------
                iB = sb
                nu = 2 * iB + 2        # jj pairs incl. the diagonal pair
                anorms = [None] * 4
                for hp in range(HG // 2):
                    h0, h1 = 2 * hp, 2 * hp + 1
                    m = hp
                    accs = {h0: ps_acc.tile([128, SB], F32, tag="acc", name=f"acc0_{iB}_{hp}"),
                            h1: ps_acc.tile([128, SB], F32, tag="acc", name=f"acc1_{iB}_{hp}")}
                    for u in range(nu):
                        jj0, jj1 = 2 * u, 2 * u + 1
                        # staircase: the last chunk pair (keys in
                        # [512iB+256, 512iB+512)) can only be attended by the
                        # top query half — compute just those 256 queries
                        half = (u == nu - 1)
                        qw = SB // 2 if half else SB     # queries per chunk
                        qoff = iB * SB + (SB // 2 if half else 0)
                        pss = {}
                        for h in (h0, h1):
                            rb = (h % 2) * 64
                            ps2 = ps_mm.tile([128, 1024], F32, tag="mm",
                                             name=f"ps_{iB}_{hp}_{u}_{h}")
                            pss[h] = ps2
                            for q, jj in ((0, jj0), (1, jj1)):
                                nc.tensor.matmul(
                                    ps2[:, q * qw:(q + 1) * qw],
                                    kt[m][rb:rb + 64, ts(jj, 128)],
                                    qt[m][rb:rb + 64, qoff:qoff + qw],
                                    start=True, stop=True,
                                )
                        nc.tensor.ldweights(junk16)
                        for h in (h0, h1):
                            e_t = e_pool.tile([128, 1024], BF16, tag="e",
                                              name=f"e_{iB}_{hp}_{u}_{h}")
                            nc.scalar.activation(e_t[:, 0:2 * qw], pss[h][:, 0:2 * qw],
                                                 AF.Exp, scale=float(SCALE))
                            t0 = jj0 - 4 * iB
                            if t0 >= 0:
                                # keep key (128*(t0+c) + p) <= query q
                                nc.gpsimd.affine_select(
                                    out=e_t[:, 0:2 * qw], in_=e_t[:, 0:2 * qw],
                                    pattern=[[-128, 2], [1, qw]],
                                    compare_op=mybir.AluOpType.is_ge,
                                    fill=0.0,
                                    base=-128 * t0 + (qoff - iB * SB),
                                    channel_multiplier=-1,
                                )
                            for q, jj in ((0, jj0), (1, jj1)):
                                nc.tensor.matmul(
                                    accs[h][0:HD + 1, qoff - iB * SB:
                                            qoff - iB * SB + qw],
                                    vt[jj][:, h, :],
                                    e_t[:, q * qw:(q + 1) * qw],
                                    start=(u == 0 and q == 0),
                                    stop=(u == nu - 1 and q == 1),
                                )
                            nc.tensor.ldweights(junk16)
                    an_pair = an_pool.tile([128, SB], F32R, tag="an",
                                           name=f"an_{iB}_{hp}")
                    anorms[hp] = an_pair
                    for h in (h0, h1):
                        t65 = rec_pool.tile([HD + 1, SB], F32, tag="t65",
                                            name=f"t65_{iB}_{h}")
                        nc.vector.tensor_copy(t65, accs[h][0:HD + 1, :])
                        r_t = rec_pool.tile([1, SB], F32, tag="r",
                                            name=f"r_{iB}_{h}")
                        nc.sync.dma_start(out=r_t, in_=t65[HD:HD + 1, :])
                        rec_t = rec_pool.tile([1, SB], F32, tag="rec",
                                              name=f"rec_{iB}_{h}")
                        nc.vector.reciprocal_approx_fast(out=rec_t, in_=r_t)
                        bc_t = bc_pool.tile([HD, SB], F32, tag="bc",
                                            name=f"bc_{iB}_{h}")
                        nc.gpsimd.partition_broadcast(bc_t, rec_t)
                        if h == h0:
                            nc.vector.tensor_mul(an_pair[0:HD, :],
                                                 t65[0:HD, :], bc_t)
                        else:
                            an_odd = an_pool.tile([HD, SB], F32R, tag="anodd",
                                                  name=f"anodd_{iB}_{h}")
                            nc.vector.tensor_mul(an_odd, t65[0:HD, :], bc_t)
                            # stage odd head to partitions 64..127 (DMA can
                            # move across partitions; DVE cannot)
                            nc.sync.dma_start(out=an_pair[HD:128, :], in_=an_odd)

                # output projection for this query block, summed over heads
                for ic in range(4):
                    o_t = o_pool.tile([128, D], F32, tag="o")
                    for dh in range(2):
                        po = ps_proj.tile([128, 512], F32, tag="po")
                        for hp2 in range(4):
                            nc.tensor.matmul(
                                po,
                                anorms[hp2][:, ts(ic, 128)],
                                wot[hp2][:, ts(dh, 512)],
                                start=(hp2 == 0), stop=(hp2 == 3),
                            )
                        nc.tensor.ldweights(junk16)
                        nc.vector.tensor_copy(o_t[:, ts(dh, 512)], po)
                    nc.sync.dma_start(
                        out=out_d[iB * SB + ic * 128:iB * SB + (ic + 1) * 128, :],
                        in_=o_t,
                    )

    nc.compile()
    return nc


def make_in_maps(x, W_q, W_k, W_v, W_o):
    x = np.asarray(x, dtype=np.float32)
    W_q = np.asarray(W_q, dtype=np.float32)
    W_k = np.asarray(W_k, dtype=np.float32)
    W_v = np.asarray(W_v, dtype=np.float32)
    W_o = np.asarray(W_o, dtype=np.float32)

    in_maps = []
    for c in range(NCORES):
        b, g = c // 2, c % 2
        cols = slice(g * DC, (g + 1) * DC)
        in_maps.append({
            "xt": np.ascontiguousarray(x[b].T).astype(BFNP),
            "wq": np.ascontiguousarray(W_q[:, cols]).astype(BFNP),
            "wk": np.ascontiguousarray(W_k[:, cols]).astype(BFNP),
            "wv": np.ascontiguousarray(W_v[:, cols]).astype(BFNP),
            "wot": np.ascontiguousarray(W_o[:, cols].T),
        })
    return in_maps


def kernel(x, W_q, W_k, W_v, W_o):
    global _cached_nc
    if _cached_nc is None:
        _cached_nc = _build()
    nc = _cached_nc

    in_maps = make_in_maps(x, W_q, W_k, W_v, W_o)
    res = run_bass_kernel_spmd(nc, in_maps, list(range(NCORES))).results
    out = np.empty((B, S, D), np.float32)
    for b in range(B):
        out[b] = res[2 * b]["out"] + res[2 * b + 1]["out"]
    return out
